# revision 11
# baseline (speedup 1.0000x reference)
"""BitLinear (ternary-quantized linear) Trainium2 kernel.

out = x @ (gamma * ternary(weight)).T + bias, computed tensor-parallel over
8 NeuronCores: weight/bias sharded along out_features, x replicated.

The device program is a pure bf16 matmul streamer: all input preparation
(gamma, ternary quantization of the weight shard, bf16 cast of x, and layout
tiling so every DMA line is long and contiguous) happens on host, where it is
exact fp32 math identical to the reference's. Per core:

  1. DMA the pre-tiled quantized weight shard (4 x 4 MiB, n-block-major, on
     the Scalar queue) and x tiles (pairs of m-tiles, on the Sync queue) into
     SBUF. Each HWDGE queue has ~20us fixed startup to first completion, so
     the first-matmul gate is one 2 MiB xt DMA + one 4 MiB wq chunk in
     parallel.
  2. 8192 bf16 128x128x512 matmuls accumulating fp32 in PSUM. The first
     m-tile pair runs nb-outer interleaved across both m-tiles, so the PE
     consumes each 4 MiB weight chunk over ~14us while the next one arrives
     (~11us) - the weight-load ramp stays PE-bound. Steady state runs
     kt-outer/nb-inner so one LDWEIGHTS (stationary x tile) serves 4 matmuls
     and is hidden by the PE's reorder window.
  3. Drain: psum * gamma on DVE (4 psum banks), + bias, DMA out on the
     Scalar queue; the final m-tile drains per n-block so the last store is
     not gated on a full 2048-wide pass.

gamma = max(mean(|clip(w, -2, 2)|), 1e-4) is computed on host with the same
jnp ops the module uses so the quantization boundary matches bit-exactly.
"""

import numpy as np
import ml_dtypes

import concourse.mybir as mybir
import concourse.tile as tile
from concourse import bacc
from concourse.bass_utils import run_bass_kernel_spmd

P = 128
B, S, D_IN, D_OUT = 4, 2048, 4096, 16384
M = B * S                 # 8192 tokens
K = D_IN                  # 4096 contraction
N_CORES = 8
NS = D_OUT // N_CORES     # 2048 out-features per core
KT = K // P               # 32 k-subtiles
MT = M // P               # 64 m-tiles
MP = MT // 2              # 32 m-tile pairs
NBS = 512                 # psum bank free size (fp32)
NB = NS // NBS            # 4 psum n-blocks

F32 = mybir.dt.float32
BF16 = mybir.dt.bfloat16

_NC_CACHE = None
LAST_RESULTS = None


def _build_nc():
    nc = bacc.Bacc(None, target_bir_lowering=False, debug=False)

    # host-tiled inputs:
    #   xt[jp][p][jj*K + kt*128 + m] = x[(2*jp+jj)*128 + m, kt*128 + p]
    #   wq[nb][p][kt*512 + n]        = ternary_w[nb*512 + n, kt*128 + p]
    xt_in = nc.declare_dram_parameter("xt", [MP, P, 2 * K], BF16, isOutput=False)
    wq_in = nc.declare_dram_parameter("wq", [NB, P, KT * NBS], BF16, isOutput=False)
    b_in = nc.declare_dram_parameter("bias", [P, NS], F32, isOutput=False)
    s_in = nc.declare_dram_parameter("scal", [P, 1], F32, isOutput=False)
    y_out = nc.declare_dram_parameter("out", [M, NS], F32, isOutput=True)

    with tile.TileContext(nc) as tc:
        with (
            tc.tile_pool(name="const", bufs=1) as constp,
            tc.tile_pool(name="xt", bufs=2) as xtp,
            tc.tile_pool(name="osb", bufs=3) as osbp,
            tc.tile_pool(name="psum", bufs=8, space="PSUM") as psump,
        ):
            wq_sb = constp.tile([P, NB, KT * NBS], BF16)
            scal = constp.tile([P, 1], F32)
            bias_sb = constp.tile([P, NS], F32)

            def drain(j, psums, pipelined):
                osb = osbp.tile([P, NS], F32, tag="osb", name=f"osb_{j}")
                if pipelined:
                    for nb in range(NB):
                        sl = slice(nb * NBS, (nb + 1) * NBS)
                        nc.vector.tensor_scalar(
                            osb[:, sl], psums[nb][:], scal[:, 0:1], None,
                            mybir.AluOpType.mult,
                        )
                        nc.vector.tensor_tensor(
                            osb[:, sl], osb[:, sl], bias_sb[:, sl],
                            mybir.AluOpType.add,
                        )
                        nc.scalar.dma_start(
                            out=y_out[j * P:(j + 1) * P, sl], in_=osb[:, sl]
                        )
                else:
                    for nb in range(NB):
                        nc.vector.tensor_scalar(
                            osb[:, nb * NBS:(nb + 1) * NBS],
                            psums[nb][:],
                            scal[:, 0:1],
                            None,
                            mybir.AluOpType.mult,
                        )
                    nc.vector.tensor_tensor(
                        osb[:], osb[:], bias_sb[:], mybir.AluOpType.add
                    )
                    nc.scalar.dma_start(
                        out=y_out[j * P:(j + 1) * P, :], in_=osb[:]
                    )

            KH = KT * NBS // 2  # wq kt-half chunk (2 MiB): 2 MiB gates keep
            # the ramp PE-bound (PE eats weights at ~290 GB/s with 2-way
            # m-tile reuse, under the ~358 GB/s a queue delivers)
            for jp in range(MP):
                xt_t = xtp.tile([P, 2 * K], BF16, tag="xt", name=f"xt_{jp}")
                if jp == 0:
                    # first x pair split per m-tile so the first matmul
                    # gates on 1 MiB, not 2
                    nc.sync.dma_start(out=xt_t[:, 0:K], in_=xt_in[0][:, 0:K])
                    nc.sync.dma_start(out=xt_t[:, K:2 * K], in_=xt_in[0][:, K:2 * K])
                    # weight chunks + bias on the Scalar queue, in 2 MiB
                    # kt-halves; scal behind xt pair 0 on Sync. (SWDGE/gpsimd
                    # is avoided: its software descriptor startup stalls the
                    # DMA path ~30us at kernel start.)
                    nc.sync.dma_start(out=scal[:], in_=s_in[:])
                    for nb in range(NB):
                        for kh in range(2):
                            nc.scalar.dma_start(
                                out=wq_sb[:, nb, kh * KH:(kh + 1) * KH],
                                in_=wq_in[nb][:, kh * KH:(kh + 1) * KH],
                            )
                    nc.scalar.dma_start(out=bias_sb[:], in_=b_in[:])
                else:
                    nc.sync.dma_start(out=xt_t[:], in_=xt_in[jp])
                psums = [
                    [
                        psump.tile([P, NBS], F32, tag="ps", name=f"ps_{jp}_{jj}_{nb}")
                        for nb in range(NB)
                    ]
                    for jj in range(2)
                ]
                if jp == 0:
                    # ramp: nb-outer, kt-half-outer, interleaved across the
                    # two m-tiles of the pair - each 2 MiB weight sub-chunk
                    # feeds ~7us of matmuls while the next (~6us) arrives
                    for nb in range(NB):
                        for kh in range(2):
                            for jj in range(2):
                                for kt in range(kh * KT // 2, (kh + 1) * KT // 2):
                                    nc.tensor.matmul(
                                        psums[jj][nb][:],
                                        xt_t[:, jj * K + kt * P:jj * K + (kt + 1) * P],
                                        wq_sb[:, nb, kt * NBS:(kt + 1) * NBS],
                                        start=(kt == 0),
                                        stop=(kt == KT - 1),
                                    )
                else:
                    for jj in range(2):
                        for kt in range(KT):
                            for nb in range(NB):
                                nc.tensor.matmul(
                                    psums[jj][nb][:],
                                    xt_t[:, jj * K + kt * P:jj * K + (kt + 1) * P],
                                    wq_sb[:, nb, kt * NBS:(kt + 1) * NBS],
                                    start=(kt == 0),
                                    stop=(kt == KT - 1),
                                )
                for jj in range(2):
                    j = 2 * jp + jj
                    drain(j, psums[jj], pipelined=(j == MT - 1))

    nc.compile()
    return nc


def _compute_gamma(weight: np.ndarray) -> np.float32:
    """Replicate the module's gamma computation bit-exactly (jnp, fp32)."""
    import jax
    import jax.numpy as jnp

    with jax.default_device(jax.devices("cpu")[0]):
        w_f32 = jnp.clip(jnp.asarray(weight, dtype=jnp.float32), -2.0, 2.0)
        gamma = jnp.maximum(jnp.mean(jnp.abs(w_f32)), 1e-4)
        return np.float32(np.asarray(gamma))


def kernel(x: np.ndarray, weight: np.ndarray, bias: np.ndarray) -> np.ndarray:
    global _NC_CACHE, LAST_RESULTS

    x2d = np.asarray(x, dtype=np.float32).reshape(M, K)
    weight = np.asarray(weight, dtype=np.float32)
    bias = np.asarray(bias, dtype=np.float32)

    gamma = _compute_gamma(weight)
    scal = np.full((P, 1), gamma, dtype=np.float32)

    # x: bf16 cast (RNE, same rounding a device-side cast would apply),
    # tiled to [jp, p(k_sub), (jj, kt, m)]
    xt = np.ascontiguousarray(
        x2d.astype(ml_dtypes.bfloat16)
        .reshape(MP, 2, P, KT, P)         # [jp, jj, m, kt, p]
        .transpose(0, 4, 1, 3, 2)         # [jp, p, jj, kt, m]
        .reshape(MP, P, 2 * K)
    )

    # ternary quantization, exact fp32 math as in the reference
    w_f32 = np.clip(weight, -2.0, 2.0)
    w_t = np.clip(np.round(w_f32 / gamma), -1.0, 1.0).astype(ml_dtypes.bfloat16)

    if _NC_CACHE is None:
        _NC_CACHE = _build_nc()
    nc = _NC_CACHE

    in_maps = []
    for i in range(N_CORES):
        wq_shard = np.ascontiguousarray(
            w_t[i * NS:(i + 1) * NS]              # [2048, 4096] ternary bf16
            .reshape(NB, NBS, KT, P)
            .transpose(0, 3, 2, 1)                # [nb, p, kt, n]
            .reshape(NB, P, KT * NBS)
        )
        b_shard = np.ascontiguousarray(
            np.broadcast_to(bias[i * NS:(i + 1) * NS], (P, NS))
        )
        in_maps.append({"xt": xt, "wq": wq_shard, "bias": b_shard, "scal": scal})

    res = run_bass_kernel_spmd(nc, in_maps, list(range(N_CORES)))
    LAST_RESULTS = res

    out = np.concatenate([res.results[i]["out"] for i in range(N_CORES)], axis=1)
    return np.ascontiguousarray(out.reshape(B, S, D_OUT))


# revision 17
# speedup vs baseline: 1.1962x; 1.1962x over previous
"""BitLinear (ternary-quantized linear) Trainium2 kernel.

out = x @ (gamma * ternary(weight)).T + bias, computed tensor-parallel over
8 NeuronCores: weight/bias sharded along out_features, x replicated.

The device program is a pure bf16 matmul streamer: all input preparation
(gamma, ternary quantization of the weight shard, bf16 cast of x, and layout
tiling so every DMA line is long and contiguous) happens on host, where it is
exact fp32 math identical to the reference's. Per core:

  1. DMA the pre-tiled quantized weight shard (4 x 4 MiB, n-block-major, on
     the Scalar queue) and x tiles (pairs of m-tiles, on the Sync queue) into
     SBUF. Each HWDGE queue has ~20us fixed startup to first completion, so
     the first-matmul gate is one 2 MiB xt DMA + one 4 MiB wq chunk in
     parallel.
  2. 8192 bf16 128x128x512 matmuls accumulating fp32 in PSUM. The first
     m-tile pair runs nb-outer interleaved across both m-tiles, so the PE
     consumes each 4 MiB weight chunk over ~14us while the next one arrives
     (~11us) - the weight-load ramp stays PE-bound. Steady state runs
     kt-outer/nb-inner so one LDWEIGHTS (stationary x tile) serves 4 matmuls
     and is hidden by the PE's reorder window.
  3. Drain: psum * gamma on DVE (4 psum banks), + bias, DMA out on the
     Scalar queue; the final m-tile drains per n-block so the last store is
     not gated on a full 2048-wide pass.

gamma = max(mean(|clip(w, -2, 2)|), 1e-4) is computed on host with the same
jnp ops the module uses so the quantization boundary matches bit-exactly.
"""

import numpy as np
import ml_dtypes

import concourse.mybir as mybir
import concourse.tile as tile
from concourse import bacc
from concourse.bass_utils import run_bass_kernel_spmd

P = 128
B, S, D_IN, D_OUT = 4, 2048, 4096, 16384
M = B * S                 # 8192 tokens
K = D_IN                  # 4096 contraction
N_CORES = 8
NS = D_OUT // N_CORES     # 2048 out-features per core
KT = K // P               # 32 k-subtiles
MT = M // P               # 64 m-tiles
MP = MT // 2              # 32 m-tile pairs
NBS = 512                 # psum bank free size (fp32)
NB = NS // NBS            # 4 psum n-blocks

F32 = mybir.dt.float32
BF16 = mybir.dt.bfloat16

_NC_CACHE = None
LAST_RESULTS = None


def _build_nc():
    nc = bacc.Bacc(None, target_bir_lowering=False, debug=False)

    # host-tiled inputs:
    #   xt[jp][p][jj*K + kt*128 + m] = x[(2*jp+jj)*128 + m, kt*128 + p]
    #   wq[nb][p][kt*512 + n]        = ternary_w[nb*512 + n, kt*128 + p]
    xt_in = nc.declare_dram_parameter("xt", [MP, P, 2 * K], BF16, isOutput=False)
    wq_in = nc.declare_dram_parameter("wq", [NB, P, KT * NBS], BF16, isOutput=False)
    b_in = nc.declare_dram_parameter("bias", [P, NS], F32, isOutput=False)
    s_in = nc.declare_dram_parameter("scal", [P, 1], F32, isOutput=False)
    y_out = nc.declare_dram_parameter("out", [M, NS], F32, isOutput=True)

    with tile.TileContext(nc) as tc:
        with (
            tc.tile_pool(name="const", bufs=1) as constp,
            tc.tile_pool(name="xt", bufs=2) as xtp,
            tc.tile_pool(name="osb", bufs=3) as osbp,
            tc.tile_pool(name="psum", bufs=8, space="PSUM") as psump,
        ):
            wq_sb = constp.tile([P, NB, KT * NBS], BF16)
            scal = constp.tile([P, 1], F32)
            bias_sb = constp.tile([P, NS], F32)

            def drain(j, psums, pipelined):
                osb = osbp.tile([P, NS], F32, tag="osb", name=f"osb_{j}")
                if pipelined:
                    for nb in range(NB):
                        sl = slice(nb * NBS, (nb + 1) * NBS)
                        nc.vector.tensor_scalar(
                            osb[:, sl], psums[nb][:], scal[:, 0:1], None,
                            mybir.AluOpType.mult,
                        )
                        nc.vector.tensor_tensor(
                            osb[:, sl], osb[:, sl], bias_sb[:, sl],
                            mybir.AluOpType.add,
                        )
                        nc.scalar.dma_start(
                            out=y_out[j * P:(j + 1) * P, sl], in_=osb[:, sl]
                        )
                else:
                    for nb in range(NB):
                        nc.vector.tensor_scalar(
                            osb[:, nb * NBS:(nb + 1) * NBS],
                            psums[nb][:],
                            scal[:, 0:1],
                            None,
                            mybir.AluOpType.mult,
                        )
                    nc.vector.tensor_tensor(
                        osb[:], osb[:], bias_sb[:], mybir.AluOpType.add
                    )
                    nc.scalar.dma_start(
                        out=y_out[j * P:(j + 1) * P, :], in_=osb[:]
                    )

            KH = KT * NBS // 2  # wq kt-half chunk (2 MiB): 2 MiB gates keep
            # the ramp PE-bound (PE eats weights at ~290 GB/s with 2-way
            # m-tile reuse, under the ~358 GB/s a queue delivers). Finer
            # sub-chunking (1 MiB quarters) raced: a matmul could read a
            # sub-chunk before its DMA landed - do not go below halves.
            for jp in range(MP):
                xt_t = xtp.tile([P, 2 * K], BF16, tag="xt", name=f"xt_{jp}")
                if jp == 0:
                    # first x pair split per m-tile so the first matmul
                    # gates on 1 MiB, not 2
                    nc.sync.dma_start(out=xt_t[:, 0:K], in_=xt_in[0][:, 0:K])
                    nc.sync.dma_start(out=xt_t[:, K:2 * K], in_=xt_in[0][:, K:2 * K])
                    # weight chunks + bias on the Scalar queue, in 2 MiB
                    # kt-halves; scal behind xt pair 0 on Sync. (SWDGE/gpsimd
                    # is avoided: its software descriptor startup stalls the
                    # DMA path ~30us at kernel start.)
                    nc.sync.dma_start(out=scal[:], in_=s_in[:])
                    for nb in range(NB):
                        for kh in range(2):
                            nc.scalar.dma_start(
                                out=wq_sb[:, nb, kh * KH:(kh + 1) * KH],
                                in_=wq_in[nb][:, kh * KH:(kh + 1) * KH],
                            )
                    nc.scalar.dma_start(out=bias_sb[:], in_=b_in[:])
                else:
                    nc.sync.dma_start(out=xt_t[:], in_=xt_in[jp])
                psums = [
                    [
                        psump.tile([P, NBS], F32, tag="ps", name=f"ps_{jp}_{jj}_{nb}")
                        for nb in range(NB)
                    ]
                    for jj in range(2)
                ]
                if jp == 0:
                    # ramp: nb-outer, kt-half-outer, interleaved across the
                    # two m-tiles of the pair - each 2 MiB weight sub-chunk
                    # feeds ~7us of matmuls while the next (~6us) arrives
                    for nb in range(NB):
                        for kh in range(2):
                            for jj in range(2):
                                for kt in range(kh * KT // 2, (kh + 1) * KT // 2):
                                    nc.tensor.matmul(
                                        psums[jj][nb][:],
                                        xt_t[:, jj * K + kt * P:jj * K + (kt + 1) * P],
                                        wq_sb[:, nb, kt * NBS:(kt + 1) * NBS],
                                        start=(kt == 0),
                                        stop=(kt == KT - 1),
                                    )
                elif jp < MP - 1:
                    for jj in range(2):
                        for kt in range(KT):
                            for nb in range(NB):
                                nc.tensor.matmul(
                                    psums[jj][nb][:],
                                    xt_t[:, jj * K + kt * P:jj * K + (kt + 1) * P],
                                    wq_sb[:, nb, kt * NBS:(kt + 1) * NBS],
                                    start=(kt == 0),
                                    stop=(kt == KT - 1),
                                )
                else:
                    # last pair: m-tile 62 in steady order, m-tile 63
                    # nb-outer with each n-block drained and stored as soon
                    # as it completes, so only one 512-wide drain chain
                    # trails the final matmul
                    for kt in range(KT):
                        for nb in range(NB):
                            nc.tensor.matmul(
                                psums[0][nb][:],
                                xt_t[:, kt * P:(kt + 1) * P],
                                wq_sb[:, nb, kt * NBS:(kt + 1) * NBS],
                                start=(kt == 0),
                                stop=(kt == KT - 1),
                            )
                    drain(2 * jp, psums[0], pipelined=False)
                    osb63 = osbp.tile([P, NS], F32, tag="osb", name="osb_63")
                    for nb in range(NB):
                        for kt in range(KT):
                            nc.tensor.matmul(
                                psums[1][nb][:],
                                xt_t[:, K + kt * P:K + (kt + 1) * P],
                                wq_sb[:, nb, kt * NBS:(kt + 1) * NBS],
                                start=(kt == 0),
                                stop=(kt == KT - 1),
                            )
                        sl = slice(nb * NBS, (nb + 1) * NBS)
                        nc.vector.tensor_scalar(
                            osb63[:, sl], psums[1][nb][:], scal[:, 0:1], None,
                            mybir.AluOpType.mult,
                        )
                        nc.vector.tensor_tensor(
                            osb63[:, sl], osb63[:, sl], bias_sb[:, sl],
                            mybir.AluOpType.add,
                        )
                        nc.scalar.dma_start(
                            out=y_out[(MT - 1) * P:MT * P, sl], in_=osb63[:, sl]
                        )
                if jp != MP - 1:
                    for jj in range(2):
                        drain(2 * jp + jj, psums[jj], pipelined=False)

    nc.compile()
    return nc


def _compute_gamma(weight: np.ndarray) -> np.float32:
    """Replicate the module's gamma computation bit-exactly (jnp, fp32)."""
    import jax
    import jax.numpy as jnp

    with jax.default_device(jax.devices("cpu")[0]):
        w_f32 = jnp.clip(jnp.asarray(weight, dtype=jnp.float32), -2.0, 2.0)
        gamma = jnp.maximum(jnp.mean(jnp.abs(w_f32)), 1e-4)
        return np.float32(np.asarray(gamma))


def kernel(x: np.ndarray, weight: np.ndarray, bias: np.ndarray) -> np.ndarray:
    global _NC_CACHE, LAST_RESULTS

    x2d = np.asarray(x, dtype=np.float32).reshape(M, K)
    weight = np.asarray(weight, dtype=np.float32)
    bias = np.asarray(bias, dtype=np.float32)

    gamma = _compute_gamma(weight)
    scal = np.full((P, 1), gamma, dtype=np.float32)

    # x: bf16 cast (RNE, same rounding a device-side cast would apply),
    # tiled to [jp, p(k_sub), (jj, kt, m)]
    xt = np.ascontiguousarray(
        x2d.astype(ml_dtypes.bfloat16)
        .reshape(MP, 2, P, KT, P)         # [jp, jj, m, kt, p]
        .transpose(0, 4, 1, 3, 2)         # [jp, p, jj, kt, m]
        .reshape(MP, P, 2 * K)
    )

    # ternary quantization, exact fp32 math as in the reference
    w_f32 = np.clip(weight, -2.0, 2.0)
    w_t = np.clip(np.round(w_f32 / gamma), -1.0, 1.0).astype(ml_dtypes.bfloat16)

    if _NC_CACHE is None:
        _NC_CACHE = _build_nc()
    nc = _NC_CACHE

    in_maps = []
    for i in range(N_CORES):
        wq_shard = np.ascontiguousarray(
            w_t[i * NS:(i + 1) * NS]              # [2048, 4096] ternary bf16
            .reshape(NB, NBS, KT, P)
            .transpose(0, 3, 2, 1)                # [nb, p, kt, n]
            .reshape(NB, P, KT * NBS)
        )
        b_shard = np.ascontiguousarray(
            np.broadcast_to(bias[i * NS:(i + 1) * NS], (P, NS))
        )
        in_maps.append({"xt": xt, "wq": wq_shard, "bias": b_shard, "scal": scal})

    res = run_bass_kernel_spmd(nc, in_maps, list(range(N_CORES)))
    LAST_RESULTS = res

    out = np.concatenate([res.results[i]["out"] for i in range(N_CORES)], axis=1)
    return np.ascontiguousarray(out.reshape(B, S, D_OUT))


# revision 23
# speedup vs baseline: 1.2009x; 1.0039x over previous
"""BitLinear (ternary-quantized linear) Trainium2 kernel.

out = x @ (gamma * ternary(weight)).T + bias, computed tensor-parallel over
8 NeuronCores: weight/bias sharded along out_features, x replicated.

The device program is a pure bf16 matmul streamer: all input preparation
(gamma, ternary quantization of the weight shard, bf16 cast of x, and layout
tiling so every DMA line is long and contiguous) happens on host, where it is
exact fp32 math identical to the reference's. Per core:

  1. DMA the pre-tiled quantized weight shard (4 x 4 MiB, n-block-major, on
     the Scalar queue) and x tiles (pairs of m-tiles, on the Sync queue) into
     SBUF. Each HWDGE queue has ~20us fixed startup to first completion, so
     the first-matmul gate is one 2 MiB xt DMA + one 4 MiB wq chunk in
     parallel.
  2. 8192 bf16 128x128x512 matmuls accumulating fp32 in PSUM. The first
     m-tile pair runs nb-outer interleaved across both m-tiles, so the PE
     consumes each 4 MiB weight chunk over ~14us while the next one arrives
     (~11us) - the weight-load ramp stays PE-bound. Steady state runs
     kt-outer/nb-inner so one LDWEIGHTS (stationary x tile) serves 4 matmuls
     and is hidden by the PE's reorder window.
  3. Drain: psum * gamma on DVE (4 psum banks), + bias, DMA out on the
     Scalar queue; the final m-tile drains per n-block so the last store is
     not gated on a full 2048-wide pass.

gamma = max(mean(|clip(w, -2, 2)|), 1e-4) is computed on host with the same
jnp ops the module uses so the quantization boundary matches bit-exactly.
"""

import numpy as np
import ml_dtypes

import concourse.mybir as mybir
import concourse.tile as tile
from concourse import bacc
from concourse.bass_utils import run_bass_kernel_spmd

P = 128
B, S, D_IN, D_OUT = 4, 2048, 4096, 16384
M = B * S                 # 8192 tokens
K = D_IN                  # 4096 contraction
N_CORES = 8
NS = D_OUT // N_CORES     # 2048 out-features per core
KT = K // P               # 32 k-subtiles
MT = M // P               # 64 m-tiles
MP = MT // 2              # 32 m-tile pairs
NBS = 512                 # psum bank free size (fp32)
NB = NS // NBS            # 4 psum n-blocks

F32 = mybir.dt.float32
BF16 = mybir.dt.bfloat16

_NC_CACHE = None
LAST_RESULTS = None


def _build_nc():
    nc = bacc.Bacc(None, target_bir_lowering=False, debug=False)

    # host-tiled inputs:
    #   xt[jp][p][jj*K + kt*128 + m] = x[(2*jp+jj)*128 + m, kt*128 + p]
    #   wq[nb][p][kt*512 + n]        = ternary_w[nb*512 + n, kt*128 + p]
    xt_in = nc.declare_dram_parameter("xt", [MP, P, 2 * K], BF16, isOutput=False)
    wq_in = nc.declare_dram_parameter("wq", [NB, P, KT * NBS], BF16, isOutput=False)
    b_in = nc.declare_dram_parameter("bias", [P, NS], F32, isOutput=False)
    s_in = nc.declare_dram_parameter("scal", [P, 1], F32, isOutput=False)
    y_out = nc.declare_dram_parameter("out", [M, NS], F32, isOutput=True)

    with tile.TileContext(nc) as tc:
        with (
            tc.tile_pool(name="const", bufs=1) as constp,
            tc.tile_pool(name="xt", bufs=2) as xtp,
            tc.tile_pool(name="osb", bufs=3) as osbp,
            tc.tile_pool(name="psum", bufs=8, space="PSUM") as psump,
        ):
            # weight shard as 16 separate 1 MiB tiles (one per (nb, kt-
            # quarter), one DMA each): single-writer tiles are the
            # categorically race-free dependency class (multi-writer tile
            # regions raced - see docstring), and 1 MiB gates keep the
            # weight-load ramp PE-bound from the first matmul (~17us)
            KQT = KT // 4   # 8 k-subtiles per chunk tile
            wq_ch = [
                [
                    constp.tile([P, KQT * NBS], BF16, name=f"wq_{nb}_{kq}")
                    for kq in range(4)
                ]
                for nb in range(NB)
            ]
            scal = constp.tile([P, 1], F32)
            bias_sb = constp.tile([P, NS], F32)

            def wq_rhs(nb, kt):
                return wq_ch[nb][kt // KQT][:, (kt % KQT) * NBS:(kt % KQT + 1) * NBS]

            def drain(j, psums, pipelined):
                osb = osbp.tile([P, NS], F32, tag="osb", name=f"osb_{j}")
                if pipelined:
                    for nb in range(NB):
                        sl = slice(nb * NBS, (nb + 1) * NBS)
                        nc.vector.tensor_scalar(
                            osb[:, sl], psums[nb][:], scal[:, 0:1], None,
                            mybir.AluOpType.mult,
                        )
                        nc.vector.tensor_tensor(
                            osb[:, sl], osb[:, sl], bias_sb[:, sl],
                            mybir.AluOpType.add,
                        )
                        nc.scalar.dma_start(
                            out=y_out[j * P:(j + 1) * P, sl], in_=osb[:, sl]
                        )
                else:
                    for nb in range(NB):
                        nc.vector.tensor_scalar(
                            osb[:, nb * NBS:(nb + 1) * NBS],
                            psums[nb][:],
                            scal[:, 0:1],
                            None,
                            mybir.AluOpType.mult,
                        )
                    nc.vector.tensor_tensor(
                        osb[:], osb[:], bias_sb[:], mybir.AluOpType.add
                    )
                    nc.scalar.dma_start(
                        out=y_out[j * P:(j + 1) * P, :], in_=osb[:]
                    )

            for jp in range(MP):
                xt_t = xtp.tile([P, 2 * K], BF16, tag="xt", name=f"xt_{jp}")
                if jp == 0:
                    # first x pair split per m-tile so the first matmul
                    # gates on 1 MiB, not 2
                    nc.sync.dma_start(out=xt_t[:, 0:K], in_=xt_in[0][:, 0:K])
                    nc.sync.dma_start(out=xt_t[:, K:2 * K], in_=xt_in[0][:, K:2 * K])
                    # weight chunk tiles + bias on the Scalar queue; scal
                    # behind xt pair 0 on Sync. (SWDGE/gpsimd is avoided:
                    # its software descriptor startup stalls the DMA path
                    # ~30us at kernel start.)
                    nc.sync.dma_start(out=scal[:], in_=s_in[:])
                    for nb in range(NB):
                        for kq in range(4):
                            nc.scalar.dma_start(
                                out=wq_ch[nb][kq][:],
                                in_=wq_in[nb][:, kq * KQT * NBS:(kq + 1) * KQT * NBS],
                            )
                    nc.scalar.dma_start(out=bias_sb[:], in_=b_in[:])
                else:
                    nc.sync.dma_start(out=xt_t[:], in_=xt_in[jp])
                psums = [
                    [
                        psump.tile([P, NBS], F32, tag="ps", name=f"ps_{jp}_{jj}_{nb}")
                        for nb in range(NB)
                    ]
                    for jj in range(2)
                ]
                if jp == 0:
                    # ramp: nb-outer, kt-quarter-outer, interleaved across
                    # the two m-tiles of the pair - each 1 MiB weight chunk
                    # feeds ~3.5us of matmuls while the next (~2.9us)
                    # arrives, so the ramp is PE-bound end to end
                    for nb in range(NB):
                        for kq in range(4):
                            for jj in range(2):
                                for kt in range(kq * KQT, (kq + 1) * KQT):
                                    nc.tensor.matmul(
                                        psums[jj][nb][:],
                                        xt_t[:, jj * K + kt * P:jj * K + (kt + 1) * P],
                                        wq_rhs(nb, kt),
                                        start=(kt == 0),
                                        stop=(kt == KT - 1),
                                    )
                elif jp < MP - 1:
                    for jj in range(2):
                        for kt in range(KT):
                            for nb in range(NB):
                                nc.tensor.matmul(
                                    psums[jj][nb][:],
                                    xt_t[:, jj * K + kt * P:jj * K + (kt + 1) * P],
                                    wq_rhs(nb, kt),
                                    start=(kt == 0),
                                    stop=(kt == KT - 1),
                                )
                else:
                    # last pair: m-tile 62 in steady order, m-tile 63
                    # nb-outer with each n-block drained and stored as soon
                    # as it completes, so only one 512-wide drain chain
                    # trails the final matmul
                    for kt in range(KT):
                        for nb in range(NB):
                            nc.tensor.matmul(
                                psums[0][nb][:],
                                xt_t[:, kt * P:(kt + 1) * P],
                                wq_rhs(nb, kt),
                                start=(kt == 0),
                                stop=(kt == KT - 1),
                            )
                    drain(2 * jp, psums[0], pipelined=False)
                    osb63 = osbp.tile([P, NS], F32, tag="osb", name="osb_63")
                    for nb in range(NB):
                        for kt in range(KT):
                            nc.tensor.matmul(
                                psums[1][nb][:],
                                xt_t[:, K + kt * P:K + (kt + 1) * P],
                                wq_rhs(nb, kt),
                                start=(kt == 0),
                                stop=(kt == KT - 1),
                            )
                        sl = slice(nb * NBS, (nb + 1) * NBS)
                        nc.vector.tensor_scalar(
                            osb63[:, sl], psums[1][nb][:], scal[:, 0:1], None,
                            mybir.AluOpType.mult,
                        )
                        nc.vector.tensor_tensor(
                            osb63[:, sl], osb63[:, sl], bias_sb[:, sl],
                            mybir.AluOpType.add,
                        )
                        nc.scalar.dma_start(
                            out=y_out[(MT - 1) * P:MT * P, sl], in_=osb63[:, sl]
                        )
                if jp != MP - 1:
                    for jj in range(2):
                        drain(2 * jp + jj, psums[jj], pipelined=False)

    nc.compile()
    return nc


def _compute_gamma(weight: np.ndarray) -> np.float32:
    """Replicate the module's gamma computation bit-exactly (jnp, fp32)."""
    import jax
    import jax.numpy as jnp

    with jax.default_device(jax.devices("cpu")[0]):
        w_f32 = jnp.clip(jnp.asarray(weight, dtype=jnp.float32), -2.0, 2.0)
        gamma = jnp.maximum(jnp.mean(jnp.abs(w_f32)), 1e-4)
        return np.float32(np.asarray(gamma))


def kernel(x: np.ndarray, weight: np.ndarray, bias: np.ndarray) -> np.ndarray:
    global _NC_CACHE, LAST_RESULTS

    x2d = np.asarray(x, dtype=np.float32).reshape(M, K)
    weight = np.asarray(weight, dtype=np.float32)
    bias = np.asarray(bias, dtype=np.float32)

    gamma = _compute_gamma(weight)
    scal = np.full((P, 1), gamma, dtype=np.float32)

    # x: bf16 cast (RNE, same rounding a device-side cast would apply),
    # tiled to [jp, p(k_sub), (jj, kt, m)]
    xt = np.ascontiguousarray(
        x2d.astype(ml_dtypes.bfloat16)
        .reshape(MP, 2, P, KT, P)         # [jp, jj, m, kt, p]
        .transpose(0, 4, 1, 3, 2)         # [jp, p, jj, kt, m]
        .reshape(MP, P, 2 * K)
    )

    # ternary quantization, exact fp32 math as in the reference
    w_f32 = np.clip(weight, -2.0, 2.0)
    w_t = np.clip(np.round(w_f32 / gamma), -1.0, 1.0).astype(ml_dtypes.bfloat16)

    if _NC_CACHE is None:
        _NC_CACHE = _build_nc()
    nc = _NC_CACHE

    in_maps = []
    for i in range(N_CORES):
        wq_shard = np.ascontiguousarray(
            w_t[i * NS:(i + 1) * NS]              # [2048, 4096] ternary bf16
            .reshape(NB, NBS, KT, P)
            .transpose(0, 3, 2, 1)                # [nb, p, kt, n]
            .reshape(NB, P, KT * NBS)
        )
        b_shard = np.ascontiguousarray(
            np.broadcast_to(bias[i * NS:(i + 1) * NS], (P, NS))
        )
        in_maps.append({"xt": xt, "wq": wq_shard, "bias": b_shard, "scal": scal})

    res = run_bass_kernel_spmd(nc, in_maps, list(range(N_CORES)))
    LAST_RESULTS = res

    out = np.concatenate([res.results[i]["out"] for i in range(N_CORES)], axis=1)
    return np.ascontiguousarray(out.reshape(B, S, D_OUT))
